# revision 6
# baseline (speedup 1.0000x reference)
"""Trainium2 Bass kernel for nn_Blur: depthwise 4x4 FIR blur (upfirdn2d, pad=(2,1)).

Full inputs: input (16,256,128,128) f32, kernel (4,4) f32.
out[n,c,i,j] = sum_{p,q} K[p,q] * x[n,c,i+1-p,j+1-q]   (zero-padded)

Strategy (per core, pure data parallel over the 4096 (n,c) slices):
  The separable kernel K = f g^T turns the blur into two banded matmuls
  per slice:  O = Mh^T X Mw  with banded 128x128 Mh/Mw built on host.
  Using the image slice as the *stationary* matmul operand flips the
  partition dim each time, so no transposes are needed:
      V = mm(lhsT=X_s, rhs=Mh) = X^T Mh        [W  x H']
      O = mm(lhsT=V_s, rhs=Mw) = Mh^T X Mw     [H' x W']
  Host pre-transposes each core's shard to [H, S, W] so every DMA moves
  contiguous 8KB per partition.

Self-contained: hardcodes shapes/sharding for this problem.
"""

import numpy as np

import concourse.bass as bass  # noqa: F401  (bass must import before tile)
import concourse.mybir as mybir
from concourse import bacc, tile
from concourse.bass_utils import run_bass_kernel_spmd

N_CORES = 8
H = W = 128
N_FULL, C_FULL = 16, 256
S_TOTAL = N_FULL * C_FULL          # 4096 independent (n,c) slices
S_PER_CORE = S_TOTAL // N_CORES    # 512
KS = 4                             # FIR kernel size
SUPER = 16                         # slices per DMA batch (1 MiB per dma_start)
QUAD = 4                           # slices per PSUM group

# "f32"  : exact fp32 matmuls (4 cyc/row on PE)
# "f32r" : fp32r matmuls with duplicated moving operand (N=256 -> 1 cyc/row)
VARIANT = "f32"

_BUILD_CACHE = {}


def _filter_taps(k4):
    """SVD-split the 4x4 kernel into separable rank terms (f_r, g_r)."""
    u, s, vt = np.linalg.svd(np.asarray(k4, dtype=np.float64))
    terms = []
    for r in range(KS):
        if s[r] > s[0] * 1e-7:
            terms.append((u[:, r] * s[r], vt[r, :]))
    return terms


def _band_matrix(taps, dup):
    """[128, dup*128] banded matrix M with M[r, i] = taps[i+1-r]."""
    m = np.zeros((H, H), dtype=np.float64)
    for i in range(H):
        for p in range(KS):
            r = i + 1 - p
            if 0 <= r < H:
                m[r, i] = taps[p]
    m = m.astype(np.float32)
    if dup > 1:
        m = np.concatenate([m] * dup, axis=1)
    return np.ascontiguousarray(m)


def _build(variant, n_ranks, repeat=1):
    """Build + compile the per-core Bass program. Returns (nc, meta).

    repeat>1 wraps the whole pass in an on-device For_i loop (same data
    re-processed; used only for slope-based HW timing in test.py).
    """
    key = (variant, n_ranks, repeat)
    if key in _BUILD_CACHE:
        return _BUILD_CACHE[key]

    dup = 2 if variant == "f32r" else 1
    mmdt = mybir.dt.float32r if variant == "f32r" else mybir.dt.float32
    f32 = mybir.dt.float32

    nc = bacc.Bacc("TRN2", target_bir_lowering=False, debug=False,
                   num_devices=N_CORES)

    x = nc.dram_tensor("x", [H, S_PER_CORE, W], mmdt, kind="ExternalInput")
    mhs = [nc.dram_tensor(f"mh{r}", [H, dup * H], mmdt, kind="ExternalInput")
           for r in range(n_ranks)]
    mws = [nc.dram_tensor(f"mw{r}", [H, dup * H], mmdt, kind="ExternalInput")
           for r in range(n_ranks)]
    y = nc.dram_tensor("y", [H, S_PER_CORE, W], f32, kind="ExternalOutput")

    n_batches = S_PER_CORE // SUPER
    psum_bufs = 2 if dup == 2 else 3

    with tile.TileContext(nc) as tc:
        with (
            tc.tile_pool(name="consts", bufs=1) as cpool,
            tc.tile_pool(name="xin", bufs=4) as xpool,
            tc.tile_pool(name="vmid", bufs=4) as vpool,
            tc.tile_pool(name="yout", bufs=4) as ypool,
            tc.tile_pool(name="pv", bufs=psum_bufs, space="PSUM") as pvpool,
            tc.tile_pool(name="po", bufs=psum_bufs, space="PSUM") as popool,
        ):
            mh_sb, mw_sb = [], []
            for r in range(n_ranks):
                t = cpool.tile([H, dup * H], mmdt, tag=f"mh{r}")
                nc.sync.dma_start(out=t, in_=mhs[r][:, :])
                mh_sb.append(t)
                t = cpool.tile([H, dup * H], mmdt, tag=f"mw{r}")
                nc.sync.dma_start(out=t, in_=mws[r][:, :])
                mw_sb.append(t)

            def _body():
                for b in range(n_batches):
                    xt = xpool.tile([H, SUPER, W], mmdt)
                    nc.sync.dma_start(out=xt,
                                      in_=x[:, b * SUPER:(b + 1) * SUPER, :])
                    yt = ypool.tile([H, SUPER, W], f32)

                    for q in range(SUPER // QUAD):
                        po = popool.tile([H, QUAD * dup * H], f32)
                        for r in range(n_ranks):
                            pv = pvpool.tile([H, QUAD * dup * H], f32)
                            for s in range(QUAD):
                                sl = q * QUAD + s
                                nc.tensor.matmul(
                                    out=pv[:, s * dup * H:(s * dup + dup) * H],
                                    lhsT=xt[:, sl, :],
                                    rhs=mh_sb[r][:, :],
                                    start=True, stop=True,
                                )
                            vt = vpool.tile([H, QUAD, H], mmdt)
                            if dup > 1:
                                pv_v = pv.rearrange("p (s d w) -> p s d w",
                                                    s=QUAD, d=dup)[:, :, 0, :]
                            else:
                                pv_v = pv.rearrange("p (s w) -> p s w", s=QUAD)
                            nc.vector.tensor_copy(out=vt[:, :, :], in_=pv_v)
                            for s in range(QUAD):
                                nc.tensor.matmul(
                                    out=po[:, s * dup * H:(s * dup + dup) * H],
                                    lhsT=vt[:, s, :],
                                    rhs=mw_sb[r][:, :],
                                    start=(r == 0), stop=(r == n_ranks - 1),
                                )
                        if dup > 1:
                            po_v = po.rearrange("p (s d w) -> p s d w",
                                                s=QUAD, d=dup)[:, :, 0, :]
                        else:
                            po_v = po.rearrange("p (s w) -> p s w", s=QUAD)
                        nc.scalar.copy(out=yt[:, q * QUAD:(q + 1) * QUAD, :],
                                       in_=po_v)

                    nc.sync.dma_start(out=y[:, b * SUPER:(b + 1) * SUPER, :],
                                      in_=yt)

            if repeat > 1:
                with tc.For_i(0, repeat, 1):
                    _body()
            else:
                _body()

    nc.compile()
    _BUILD_CACHE[key] = (nc, dup)
    return nc, dup


def prepare_in_maps(input, kernel, variant=VARIANT):
    """Shard + host-transpose the full input; build band matrices."""
    dup = 2 if variant == "f32r" else 1
    terms = _filter_taps(kernel)
    x_flat = np.asarray(input, dtype=np.float32).reshape(S_TOTAL, H, W)
    consts = {}
    for r, (f, g) in enumerate(terms):
        consts[f"mh{r}"] = _band_matrix(f, dup)
        consts[f"mw{r}"] = _band_matrix(g, dup)
    in_maps = []
    for c in range(N_CORES):
        shard = x_flat[c * S_PER_CORE:(c + 1) * S_PER_CORE]  # [S, H, W]
        xh = np.ascontiguousarray(shard.transpose(1, 0, 2))  # [H, S, W]
        in_maps.append({"x": xh, **consts})
    return in_maps, len(terms)


def assemble_output(results):
    """Per-core y [H, S, W] -> full (16, 256, 128, 128)."""
    outs = []
    for c in range(N_CORES):
        yh = results[c]["y"]                                  # [H, S, W]
        outs.append(yh.transpose(1, 0, 2))                    # [S, H, W]
    out = np.concatenate(outs, axis=0)
    return np.ascontiguousarray(out.reshape(N_FULL, C_FULL, H, W))


def kernel(input, kernel):
    in_maps, n_ranks = prepare_in_maps(input, kernel, VARIANT)
    nc, _ = _build(VARIANT, n_ranks)
    res = run_bass_kernel_spmd(nc, in_maps, list(range(N_CORES)))
    return assemble_output(res.results)
